# revision 1
# baseline (speedup 1.0000x reference)
"""Trainium2 Bass kernel: single-head causal attention, data-parallel over batch.

Problem: x [4096, 64, 128] f32, Wq/Wk/Wv [128, 64] f32.
  q,k,v = x @ W*;  scores = q k^T / sqrt(128); causal softmax; out = attn @ v.

Sharding: batch 4096 -> 8 cores x 512 batches. Each core loops over 32
super-tiles of 16 batches (1024 rows of x).

Per-core pipeline (bf16 matmuls, fp32 PSUM):
  1. SWDGE DMA-cast loads x tile [128, 1024] f32->bf16.
  2. 8 PE transposes -> x^T in PSUM (bf16) -> SBUF.
  3. P1: q^T,k^T = W^T x^T per batch column-blocks (parity -> partition half).
  4. P2: v pairs in native [s, h] layout (x^T pair as stationary).
  5. P3: scores^T_b = k_b q_b^T per batch into quadrant-packed PSUM.
  6. exp on ACT (PSUM->SBUF bf16), multiplicative causal mask on GPSIMD.
  7. P4: [O'|sums] = E^T.T @ [V|ones] per batch (unnormalized attn out).
  8. normalize: O = O' * recip(sums) via stride-0 broadcast tensor_tensor.
"""

import os
import numpy as np
import ml_dtypes
from contextlib import ExitStack

F32 = None  # set after imports below (keep module import cheap for host-only use)

B, T, C, H = 4096, 64, 128, 64
N_CORES = 8
ST_B = 16                    # batches per super-tile
ROWS = ST_B * T              # 1024
B_CORE = B // N_CORES        # 512
N_ST = B_CORE // ST_B        # 32

_cached = {}


def _build_nc():
    import concourse.bass as bass
    import concourse.mybir as mybir
    import concourse.tile as tile
    from concourse import bacc

    F32 = mybir.dt.float32
    BF16 = mybir.dt.bfloat16

    nc = bacc.Bacc("TRN2", target_bir_lowering=False, debug=False)
    x_d = nc.dram_tensor("x", [B_CORE * T, C], F32, kind="ExternalInput").ap()
    wq_d = nc.dram_tensor("wq", [C, H], BF16, kind="ExternalInput").ap()
    wk_d = nc.dram_tensor("wk", [C, H], BF16, kind="ExternalInput").ap()
    wv_d = nc.dram_tensor("wv", [C, H], BF16, kind="ExternalInput").ap()
    id_d = nc.dram_tensor("ident", [C, C], BF16, kind="ExternalInput").ap()
    mk_d = nc.dram_tensor("mask", [128, 512], BF16, kind="ExternalInput").ap()
    o_d = nc.dram_tensor("o", [B_CORE * T, H], F32, kind="ExternalOutput").ap()

    with tile.TileContext(nc) as tc, ExitStack() as ctx:
        sb = ctx.enter_context(tc.tile_pool(name="sb", bufs=2))
        ps = ctx.enter_context(tc.tile_pool(name="ps", bufs=1, space="PSUM"))
        psO = ctx.enter_context(tc.tile_pool(name="psO", bufs=1, space="PSUM"))
        cpool = ctx.enter_context(tc.tile_pool(name="const", bufs=1))

        wq_sb = cpool.tile([C, H], BF16, tag="wq")
        wk_sb = cpool.tile([C, H], BF16, tag="wk")
        wv_sb = cpool.tile([C, H], BF16, tag="wv")
        id_sb = cpool.tile([C, C], BF16, tag="id")
        mk_sb = cpool.tile([128, 512], BF16, tag="mk")
        nc.sync.dma_start(wq_sb[:], wq_d)
        nc.sync.dma_start(wk_sb[:], wk_d)
        nc.sync.dma_start(wv_sb[:], wv_d)
        nc.sync.dma_start(id_sb[:], id_d)
        nc.sync.dma_start(mk_sb[:], mk_d)

        xv = x_d.rearrange("(S n p) c -> S p n c", n=8, p=128)
        ov = o_d.rearrange("(S m par t) h -> S (par t) m h", m=8, par=2, t=64)

        for st in range(N_ST):
            # ---- load x (f32), cast to bf16 on GPSIMD
            x_nat = sb.tile([128, ROWS], F32, tag="x_nat")
            nc.sync.dma_start(
                x_nat[:].rearrange("p (n c) -> p n c", n=8), xv[st]
            )
            x_bf = sb.tile([128, ROWS], BF16, tag="x_bf")
            nc.gpsimd.tensor_copy(x_bf[:], x_nat[:])

            # ---- 8 PE transposes -> xT in PSUM (bf16), then copy to SBUF
            xT_ps = ps.tile([128, ROWS // 2], F32, tag="xT")
            xT_ps_bf = xT_ps[:].bitcast(BF16)
            for i in range(8):
                nc.tensor.transpose(
                    xT_ps_bf[:, 128 * i:128 * (i + 1)],
                    x_bf[:, 128 * i:128 * (i + 1)],
                    id_sb[:],
                )
            xT_sb = sb.tile([128, ROWS], BF16, tag="xT_sb")
            nc.vector.tensor_copy(xT_sb[:], xT_ps_bf)

            # ---- P1: q^T, k^T
            # bank b (cols 512b): [0:64, 0:256]=q evens, [0:64, 256:512]=k evens
            #                     [64:128, ...] odds
            qk_ps = ps.tile([128, 1024], F32, tag="qk")
            xTv = xT_sb[:].rearrange("p (m par t) -> p par m t", par=2, t=64)
            for b in range(2):
                for par in range(2):
                    for qki, w_sb in ((0, wq_sb), (1, wk_sb)):
                        nc.tensor.matmul(
                            qk_ps[64 * par:64 * par + 64,
                                  512 * b + 256 * qki:512 * b + 256 * qki + 256],
                            w_sb[:],
                            xTv[:, par, 4 * b:4 * b + 4, :],
                            start=True, stop=True, skip_group_check=True,
                            tile_position=(0, 64 * par),
                        )
            qk_sb = sb.tile([128, 1024], BF16, tag="qk_sb")
            nc.scalar.copy(qk_sb[:], qk_ps[:])

            # ---- P2: v pairs (native [s,h])
            v_ps = ps.tile([128, 512], F32, tag="v")
            for m in range(8):
                nc.tensor.matmul(
                    v_ps[:, 64 * m:64 * m + 64],
                    xT_sb[:, 128 * m:128 * m + 128],
                    wv_sb[:],
                    start=True, stop=True,
                )
            v_sb = sb.tile([128, 8 * 66], BF16, tag="v_sb")
            v_sb_v = v_sb[:].rearrange("p (m z) -> p m z", z=66)
            nc.vector.tensor_copy(
                v_sb_v[:, :, 0:64],
                v_ps[:].rearrange("p (m t) -> p m t", t=64),
            )
            nc.gpsimd.memset(v_sb_v[:, :, 64:65], 1.0)

            # ---- P3: scores^T per batch (quadrant-packed pairs)
            sc_ps = ps.tile([128, 512], F32, tag="sc")
            for j in range(16):
                m, Hh = j // 2, 64 * (j % 2)
                b, e = j // 8, (j % 8) // 2
                qcol = 512 * b + 64 * e
                kcol = 512 * b + 256 + 64 * e
                nc.tensor.matmul(
                    sc_ps[Hh:Hh + 64, 64 * m:64 * m + 64],
                    qk_sb[Hh:Hh + 64, kcol:kcol + 64],
                    qk_sb[Hh:Hh + 64, qcol:qcol + 64],
                    start=True, stop=True, skip_group_check=True,
                    tile_position=(Hh, Hh),
                )

            # ---- exp (ACT) then multiplicative causal mask (GPSIMD)
            E_raw = sb.tile([128, 512], BF16, tag="Eraw")
            nc.scalar.activation(
                E_raw[:], sc_ps[:], mybir.ActivationFunctionType.Exp
            )
            E_sb = sb.tile([128, 512], BF16, tag="E")
            nc.gpsimd.tensor_tensor(
                out=E_sb[:], in0=E_raw[:], in1=mk_sb[:],
                op=mybir.AluOpType.mult,
            )

            # ---- P4: [O' | sums] per batch
            o_ps = psO.tile([128, 1024], F32, tag="o")
            for j in range(16):
                m, Hh = j // 2, 64 * (j % 2)
                off = 512 * (m // 4) + 65 * (m % 4)
                nc.tensor.matmul(
                    o_ps[Hh:Hh + 64, off:off + 65],
                    E_sb[Hh:Hh + 64, 64 * m:64 * m + 64],
                    v_sb[Hh:Hh + 64, 66 * m:66 * m + 65],
                    start=True, stop=True, skip_group_check=True,
                    tile_position=(Hh, Hh),
                )

            # ---- normalize: O = O' * recip(sums)
            opsv = o_ps[:].rearrange("p (B x) -> p B x", B=2)[:, :, 0:260]
            opsb = opsv.rearrange("p B (m z) -> p B m z", z=65)
            r_sb = sb.tile([128, 8], F32, tag="r")
            r_v = r_sb[:].rearrange("p (B m) -> p B m", B=2)
            nc.vector.reciprocal(r_v.unsqueeze(3), opsb[:, :, :, 64:65])
            o_sb = sb.tile([128, 512], F32, tag="o_sb")
            nc.vector.tensor_tensor(
                out=o_sb[:].rearrange("p (B m t) -> p B m t", B=2, t=64),
                in0=opsb[:, :, :, 0:64],
                in1=r_v.unsqueeze(3).broadcast_to((128, 2, 4, 64)),
                op=mybir.AluOpType.mult,
            )

            # ---- DMA out
            nc.sync.dma_start(ov[st], o_sb[:].rearrange("p (m h) -> p m h", h=64))

    nc.compile()
    return nc


def _host_inputs(x, Wq, Wk, Wv):
    bf = ml_dtypes.bfloat16
    wq_bf = np.ascontiguousarray((Wq * (C ** -0.5)).astype(bf))
    wk_bf = np.ascontiguousarray(Wk.astype(bf))
    wv_bf = np.ascontiguousarray(Wv.astype(bf))
    ident = np.eye(128, dtype=bf)
    tri = np.triu(np.ones((T, T), dtype=np.float32))  # [s, t]: 1 if s <= t
    mask = np.ascontiguousarray(np.tile(tri, (2, 8)).astype(bf))
    in_maps = []
    for c in range(N_CORES):
        shard = np.ascontiguousarray(
            x[c * B_CORE:(c + 1) * B_CORE].reshape(B_CORE * T, C)
        ).astype(np.float32)
        in_maps.append({
            "x": shard, "wq": wq_bf, "wk": wk_bf, "wv": wv_bf,
            "ident": ident, "mask": mask,
        })
    return in_maps


def run(x, Wq, Wk, Wv, trace=False, **run_kwargs):
    from concourse import bass_utils

    if "nc" not in _cached:
        _cached["nc"] = _build_nc()
    nc = _cached["nc"]
    in_maps = _host_inputs(np.asarray(x), np.asarray(Wq),
                           np.asarray(Wk), np.asarray(Wv))
    res = bass_utils.run_bass_kernel_spmd(
        nc, in_maps, core_ids=list(range(N_CORES)), trace=trace, **run_kwargs
    )
    outs = [r["o"].reshape(B_CORE, T, H) for r in res.results]
    return np.concatenate(outs, axis=0), res


def kernel(x, Wq, Wk, Wv):
    out, _ = run(x, Wq, Wk, Wv, trace=False)
    return out



# revision 5
# speedup vs baseline: 1.3587x; 1.3587x over previous
"""Trainium2 Bass kernel: single-head causal attention, data-parallel over batch.

Problem: x [4096, 64, 128] f32, Wq/Wk/Wv [128, 64] f32.
  q,k,v = x @ W*;  scores = q k^T / sqrt(128); causal softmax; out = attn @ v.

Sharding: batch 4096 -> 8 cores x 512 batches. Each core loops over 32
super-tiles of 16 batches (1024 rows of x).

v2 pipeline (bf16 matmuls, fp32 PSUM):
  1. SWDGE DMA-cast loads x tile [128, 1024] f32->bf16 (cast in DMA).
  2. 8 PE transposes -> x^T in PSUM (bf16) -> SBUF.
  3. Y-pass: y^T = A^T x^T with A = (Wq/sqrt(C)) Wk^T precomputed on host
     (fuses the q- and k-projections: scores_b = x_b A x_b^T).
  4. V-pass: v pairs in native [s, h] layout (x^T pair as stationary).
  5. P3: scores^T_b = x_b y_b^T per batch: lhsT = xT_b, rhs = yT_b,
     2-way col-packed (K=128 full array).
  6. exp on ACT writes block-diag E pair slabs; multiplicative causal
     mask on GPSIMD (off-diag blocks stay zero from a one-time memset).
  7. P4: [O'|sums] = E_pair^T @ [V_pair|ones]: one M=128 matmul per
     2-batch pair (block-diagonal stationary).
  8. normalize: O = O' * recip(sums) via stride-0 broadcast tensor_tensor.
"""

import os
import numpy as np
import ml_dtypes
from contextlib import ExitStack

B, T, C, H = 4096, 64, 128, 64
N_CORES = 8
ST_B = 16                    # batches per super-tile
ROWS = ST_B * T              # 1024
B_CORE = B // N_CORES        # 512
N_ST = B_CORE // ST_B        # 32

_cached = {}


def _build_nc():
    import concourse.bass as bass
    import concourse.mybir as mybir
    import concourse.tile as tile
    from concourse import bacc

    F32 = mybir.dt.float32
    BF16 = mybir.dt.bfloat16

    nc = bacc.Bacc("TRN2", target_bir_lowering=False, debug=False)
    x_d = nc.dram_tensor("x", [B_CORE * T, C], F32, kind="ExternalInput").ap()
    a_d = nc.dram_tensor("amat", [C, C], BF16, kind="ExternalInput").ap()
    wv_d = nc.dram_tensor("wv", [C, H], BF16, kind="ExternalInput").ap()
    id_d = nc.dram_tensor("ident", [C, C], BF16, kind="ExternalInput").ap()
    mk_d = nc.dram_tensor("mask", [128, 512], BF16, kind="ExternalInput").ap()
    o_d = nc.dram_tensor("o", [B_CORE * T, H], F32, kind="ExternalOutput").ap()

    with tile.TileContext(nc) as tc, ExitStack() as ctx:
        sb = ctx.enter_context(tc.tile_pool(name="sb", bufs=2))
        ps = ctx.enter_context(tc.tile_pool(name="ps", bufs=1, space="PSUM"))
        psS = ctx.enter_context(tc.tile_pool(name="psS", bufs=2, space="PSUM"))
        psO = ctx.enter_context(tc.tile_pool(name="psO", bufs=1, space="PSUM"))
        cpool = ctx.enter_context(tc.tile_pool(name="const", bufs=1))

        a_sb = cpool.tile([C, C], BF16, tag="amat")
        wv_sb = cpool.tile([C, H], BF16, tag="wv")
        id_sb = cpool.tile([C, C], BF16, tag="id")
        mk_sb = cpool.tile([128, 512], BF16, tag="mk")
        nc.sync.dma_start(a_sb[:], a_d)
        nc.sync.dma_start(wv_sb[:], wv_d)
        nc.sync.dma_start(id_sb[:], id_d)
        nc.sync.dma_start(mk_sb[:], mk_d)

        # persistent double-buffered block-diag E slabs; off-diag stays 0
        E_bufs = [cpool.tile([128, ST_B * 128 // 2], BF16, tag=f"E{i}",
                             name=f"E{i}")
                  for i in range(2)]
        for Eb in E_bufs:
            nc.gpsimd.memset(Eb[:], 0.0)

        xv = x_d.rearrange("(S n p) c -> S p n c", n=8, p=128)
        ov = o_d.rearrange("(S m p) h -> S p m h", m=8, p=128)

        for st in range(N_ST):
            # ---- SWDGE DMA-cast load: x tile f32 -> bf16
            x_bf = sb.tile([128, ROWS], BF16, tag="x_bf")
            nc.gpsimd.dma_start(
                x_bf[:].rearrange("p (n c) -> p n c", n=8), xv[st]
            )

            # ---- 8 PE transposes -> xT in PSUM (bf16), then copy to SBUF
            xT_ps = ps.tile([128, ROWS // 2], F32, tag="xT")
            xT_ps_bf = xT_ps[:].bitcast(BF16)
            for i in range(8):
                nc.tensor.transpose(
                    xT_ps_bf[:, 128 * i:128 * (i + 1)],
                    x_bf[:, 128 * i:128 * (i + 1)],
                    id_sb[:],
                )
            xT_sb = sb.tile([128, ROWS], BF16, tag="xT_sb")
            nc.vector.tensor_copy(xT_sb[:], xT_ps_bf)

            # ---- Y-pass: y^T = A^T @ x^T  [c2, rows]
            y_ps = ps.tile([128, ROWS], F32, tag="y")
            for b in range(2):
                nc.tensor.matmul(
                    y_ps[:, 512 * b:512 * b + 512],
                    a_sb[:],
                    xT_sb[:, 512 * b:512 * b + 512],
                    start=True, stop=True,
                )
            y_sb = sb.tile([128, ROWS], BF16, tag="y_sb")
            nc.scalar.copy(y_sb[:], y_ps[:])

            # ---- V-pass: v pairs (native [s,h]), xT pair as stationary
            v_ps = ps.tile([128, 512], F32, tag="v")
            for m in range(8):
                nc.tensor.matmul(
                    v_ps[:, 64 * m:64 * m + 64],
                    xT_sb[:, 128 * m:128 * m + 128],
                    wv_sb[:],
                    start=True, stop=True,
                )
            v_sb = sb.tile([128, 8 * 66], BF16, tag="v_sb")
            v_sb_v = v_sb[:].rearrange("p (m z) -> p m z", z=66)
            nc.vector.tensor_copy(
                v_sb_v[:, :, 0:64],
                v_ps[:].rearrange("p (m t) -> p m t", t=64),
            )
            nc.gpsimd.memset(v_sb_v[:, :, 64:65], 1.0)

            # ---- P3: scores^T_b = x_b y_b^T  (K=128, 2-way col-packed)
            sc_ps = psS.tile([128, 512], F32, tag="sc")
            for b in range(ST_B):
                m, half = b // 2, b % 2
                col = 128 * m + 64 * half
                nc.tensor.matmul(
                    sc_ps[64 * half:64 * half + 64, 64 * m:64 * m + 64],
                    xT_sb[:, col:col + 64],
                    y_sb[:, col:col + 64],
                    start=True, stop=True, skip_group_check=True,
                    tile_position=(0, 64 * half),
                )

            # ---- exp into block-diag E slabs (ACT), then causal mask (GPSIMD)
            E = E_bufs[st % 2]
            Ev = E[:].rearrange("p (m w) -> p m w", w=128)
            scv = sc_ps[:].rearrange("p (m t) -> p m t", t=64)
            mkv = mk_sb[:].rearrange("p (m t) -> p m t", t=64)
            nc.scalar.activation(
                Ev[0:64, :, 0:64], scv[0:64],
                mybir.ActivationFunctionType.Exp,
            )
            nc.scalar.activation(
                Ev[64:128, :, 64:128], scv[64:128],
                mybir.ActivationFunctionType.Exp,
            )
            nc.gpsimd.tensor_tensor(
                out=Ev[0:64, :, 0:64], in0=Ev[0:64, :, 0:64],
                in1=mkv[0:64], op=mybir.AluOpType.mult,
            )
            nc.gpsimd.tensor_tensor(
                out=Ev[64:128, :, 64:128], in0=Ev[64:128, :, 64:128],
                in1=mkv[64:128], op=mybir.AluOpType.mult,
            )

            # ---- P4: [O'|sums] = E_pair^T @ [V_pair|ones], M=128 per pair
            o_ps = psO.tile([128, 1024], F32, tag="o")
            for m in range(8):
                nc.tensor.matmul(
                    o_ps[:, 128 * m:128 * m + 65],
                    E[:, 128 * m:128 * m + 128],
                    v_sb[:, 66 * m:66 * m + 65],
                    start=True, stop=True,
                )

            # ---- normalize: O = O' * recip(sums)
            opsv = o_ps[:].rearrange("p (m w) -> p m w", w=128)
            r_sb = sb.tile([128, 8], F32, tag="r")
            nc.vector.reciprocal(r_sb[:].unsqueeze(2), opsv[:, :, 64:65])
            o_sb = sb.tile([128, 512], F32, tag="o_sb")
            nc.vector.tensor_tensor(
                out=o_sb[:].rearrange("p (m h) -> p m h", h=64),
                in0=opsv[:, :, 0:64],
                in1=r_sb[:].unsqueeze(2).broadcast_to((128, 8, 64)),
                op=mybir.AluOpType.mult,
            )

            # ---- DMA out
            nc.sync.dma_start(ov[st], o_sb[:].rearrange("p (m h) -> p m h", h=64))

    nc.compile()
    return nc


def _host_inputs(x, Wq, Wk, Wv):
    bf = ml_dtypes.bfloat16
    amat = np.ascontiguousarray(
        ((Wq.astype(np.float32) * (C ** -0.5)) @ Wk.astype(np.float32).T)
        .astype(bf)
    )
    wv_bf = np.ascontiguousarray(Wv.astype(bf))
    ident = np.eye(128, dtype=bf)
    tri = np.triu(np.ones((T, T), dtype=np.float32))  # [s, t]: 1 if s <= t
    mask = np.ascontiguousarray(np.tile(tri, (2, 8)).astype(bf))
    in_maps = []
    for c in range(N_CORES):
        shard = np.ascontiguousarray(
            x[c * B_CORE:(c + 1) * B_CORE].reshape(B_CORE * T, C)
        ).astype(np.float32)
        in_maps.append({
            "x": shard, "amat": amat, "wv": wv_bf,
            "ident": ident, "mask": mask,
        })
    return in_maps


def run(x, Wq, Wk, Wv, trace=False, **run_kwargs):
    from concourse import bass_utils

    if "nc" not in _cached:
        _cached["nc"] = _build_nc()
    nc = _cached["nc"]
    in_maps = _host_inputs(np.asarray(x), np.asarray(Wq),
                           np.asarray(Wk), np.asarray(Wv))
    res = bass_utils.run_bass_kernel_spmd(
        nc, in_maps, core_ids=list(range(N_CORES)), trace=trace, **run_kwargs
    )
    outs = [r["o"].reshape(B_CORE, T, H) for r in res.results]
    return np.concatenate(outs, axis=0), res


def kernel(x, Wq, Wk, Wv):
    out, _ = run(x, Wq, Wk, Wv, trace=False)
    return out


# revision 6
# speedup vs baseline: 1.5176x; 1.1169x over previous
"""Trainium2 Bass kernel: single-head causal attention, data-parallel over batch.

Problem: x [4096, 64, 128] f32, Wq/Wk/Wv [128, 64] f32.
  q,k,v = x @ W*;  scores = q k^T / sqrt(128); causal softmax; out = attn @ v.

Sharding: batch 4096 -> 8 cores x 512 batches. Each core loops over 32
super-tiles of 16 batches (1024 rows of x).

v2 pipeline (bf16 matmuls, fp32 PSUM):
  1. SWDGE DMA-cast loads x tile [128, 1024] f32->bf16 (cast in DMA).
  2. 8 PE transposes -> x^T in PSUM (bf16) -> SBUF.
  3. Y-pass: y^T = A^T x^T with A = (Wq/sqrt(C)) Wk^T precomputed on host
     (fuses the q- and k-projections: scores_b = x_b A x_b^T).
  4. V-pass: v pairs in native [s, h] layout (x^T pair as stationary).
  5. P3: scores^T_b = x_b y_b^T per batch: lhsT = xT_b, rhs = yT_b,
     2-way col-packed (K=128 full array).
  6. exp on ACT writes block-diag E pair slabs; multiplicative causal
     mask on GPSIMD (off-diag blocks stay zero from a one-time memset).
  7. P4: [O'|sums] = E_pair^T @ [V_pair|ones]: one M=128 matmul per
     2-batch pair (block-diagonal stationary).
  8. normalize: O = O' * recip(sums) via stride-0 broadcast tensor_tensor.
"""

import os
import numpy as np
import ml_dtypes
from contextlib import ExitStack

B, T, C, H = 4096, 64, 128, 64
N_CORES = 8
ST_B = 16                    # batches per super-tile
ROWS = ST_B * T              # 1024
B_CORE = B // N_CORES        # 512
N_ST = B_CORE // ST_B        # 32

_cached = {}


def _build_nc():
    import concourse.bass as bass
    import concourse.mybir as mybir
    import concourse.tile as tile
    from concourse import bacc

    F32 = mybir.dt.float32
    BF16 = mybir.dt.bfloat16

    nc = bacc.Bacc("TRN2", target_bir_lowering=False, debug=False)
    x_d = nc.dram_tensor("x", [B_CORE * T, C], F32, kind="ExternalInput").ap()
    a_d = nc.dram_tensor("amat", [C, C], BF16, kind="ExternalInput").ap()
    wv_d = nc.dram_tensor("wv", [C, H], BF16, kind="ExternalInput").ap()
    id_d = nc.dram_tensor("ident", [C, C], BF16, kind="ExternalInput").ap()
    mk_d = nc.dram_tensor("mask", [128, 512], BF16, kind="ExternalInput").ap()
    o_d = nc.dram_tensor("o", [B_CORE * T, H], F32, kind="ExternalOutput").ap()

    with tile.TileContext(nc) as tc, ExitStack() as ctx:
        sb = ctx.enter_context(tc.tile_pool(name="sb", bufs=3))
        ps = ctx.enter_context(tc.tile_pool(name="ps", bufs=1, space="PSUM"))
        psS = ctx.enter_context(tc.tile_pool(name="psS", bufs=2, space="PSUM"))
        psO = ctx.enter_context(tc.tile_pool(name="psO", bufs=1, space="PSUM"))
        cpool = ctx.enter_context(tc.tile_pool(name="const", bufs=1))

        a_sb = cpool.tile([C, C], BF16, tag="amat")
        wv_sb = cpool.tile([C, H], BF16, tag="wv")
        id_sb = cpool.tile([C, C], BF16, tag="id")
        mk_sb = cpool.tile([128, 512], BF16, tag="mk")
        nc.sync.dma_start(a_sb[:], a_d)
        nc.sync.dma_start(wv_sb[:], wv_d)
        nc.sync.dma_start(id_sb[:], id_d)
        nc.sync.dma_start(mk_sb[:], mk_d)

        xv = x_d.rearrange("(S n p) c -> S p n c", n=8, p=128)
        ov = o_d.rearrange("(S m par t) h -> S (par t) m h", m=8, par=2, t=64)

        for st in range(N_ST):
            # ---- SWDGE DMA-cast load: x tile f32 -> bf16
            x_bf = sb.tile([128, ROWS], BF16, tag="x_bf")
            nc.gpsimd.dma_start(
                x_bf[:].rearrange("p (n c) -> p n c", n=8), xv[st]
            )

            # ---- 8 PE transposes -> xT in PSUM (bf16), then copy to SBUF
            xT_ps = ps.tile([128, ROWS // 2], F32, tag="xT")
            xT_ps_bf = xT_ps[:].bitcast(BF16)
            for i in range(8):
                nc.tensor.transpose(
                    xT_ps_bf[:, 128 * i:128 * (i + 1)],
                    x_bf[:, 128 * i:128 * (i + 1)],
                    id_sb[:],
                )
            xT_sb = sb.tile([128, ROWS], BF16, tag="xT_sb")
            nc.vector.tensor_copy(xT_sb[:], xT_ps_bf)

            # ---- Y-pass: y^T = A^T @ x^T  [c2, rows]
            y_ps = ps.tile([128, ROWS], F32, tag="y")
            for b in range(2):
                nc.tensor.matmul(
                    y_ps[:, 512 * b:512 * b + 512],
                    a_sb[:],
                    xT_sb[:, 512 * b:512 * b + 512],
                    start=True, stop=True,
                )
            y_sb = sb.tile([128, ROWS], BF16, tag="y_sb")
            nc.scalar.copy(y_sb[:], y_ps[:])

            # ---- V-pass: v pairs (native [s,h]), xT pair as stationary
            v_ps = ps.tile([128, 512], F32, tag="v")
            for m in range(8):
                nc.tensor.matmul(
                    v_ps[:, 64 * m:64 * m + 64],
                    xT_sb[:, 128 * m:128 * m + 128],
                    wv_sb[:],
                    start=True, stop=True,
                )
            v_sb = sb.tile([128, 8 * 66], BF16, tag="v_sb")
            v_sb_v = v_sb[:].rearrange("p (m z) -> p m z", z=66)
            nc.vector.tensor_copy(
                v_sb_v[:, :, 0:64],
                v_ps[:].rearrange("p (m t) -> p m t", t=64),
            )
            nc.gpsimd.memset(v_sb_v[:, :, 64:65], 1.0)

            # ---- P3: scores^T_b = x_b y_b^T  (K=128, 2-way col-packed)
            sc_ps = psS.tile([128, 512], F32, tag="sc")
            for b in range(ST_B):
                m, half = b // 2, b % 2
                col = 128 * m + 64 * half
                nc.tensor.matmul(
                    sc_ps[64 * half:64 * half + 64, 64 * m:64 * m + 64],
                    xT_sb[:, col:col + 64],
                    y_sb[:, col:col + 64],
                    start=True, stop=True, skip_group_check=True,
                    tile_position=(0, 64 * half),
                )

            # ---- exp (ACT) then multiplicative causal mask (Vector)
            E_raw = sb.tile([128, 512], BF16, tag="Eraw")
            nc.scalar.activation(
                E_raw[:], sc_ps[:], mybir.ActivationFunctionType.Exp
            )
            E_sb = sb.tile([128, 512], BF16, tag="E")
            nc.vector.tensor_tensor(
                out=E_sb[:], in0=E_raw[:], in1=mk_sb[:],
                op=mybir.AluOpType.mult,
            )

            # ---- P4: [O'|sums] = E_b^T @ [V_b|ones] per batch (quadrants)
            o_ps = psO.tile([128, 1024], F32, tag="o")
            for b in range(ST_B):
                m, half = b // 2, b % 2
                Hh = 64 * half
                off = 512 * (m // 4) + 65 * (m % 4)
                nc.tensor.matmul(
                    o_ps[Hh:Hh + 64, off:off + 65],
                    E_sb[Hh:Hh + 64, 64 * m:64 * m + 64],
                    v_sb[Hh:Hh + 64, 66 * m:66 * m + 65],
                    start=True, stop=True, skip_group_check=True,
                    tile_position=(Hh, Hh),
                )

            # ---- normalize: O = O' * recip(sums)
            opsv = o_ps[:].rearrange("p (B x) -> p B x", B=2)[:, :, 0:260]
            opsb = opsv.rearrange("p B (m z) -> p B m z", z=65)
            r_sb = sb.tile([128, 8], F32, tag="r")
            r_v = r_sb[:].rearrange("p (B m) -> p B m", B=2)
            nc.vector.reciprocal(r_v.unsqueeze(3), opsb[:, :, :, 64:65])
            o_sb = sb.tile([128, 512], F32, tag="o_sb")
            nc.vector.tensor_tensor(
                out=o_sb[:].rearrange("p (B m t) -> p B m t", B=2, t=64),
                in0=opsb[:, :, :, 0:64],
                in1=r_v.unsqueeze(3).broadcast_to((128, 2, 4, 64)),
                op=mybir.AluOpType.mult,
            )

            # ---- DMA out
            nc.sync.dma_start(ov[st], o_sb[:].rearrange("p (m h) -> p m h", h=64))

    nc.compile()
    return nc


def _host_inputs(x, Wq, Wk, Wv):
    bf = ml_dtypes.bfloat16
    amat = np.ascontiguousarray(
        ((Wq.astype(np.float32) * (C ** -0.5)) @ Wk.astype(np.float32).T)
        .astype(bf)
    )
    wv_bf = np.ascontiguousarray(Wv.astype(bf))
    ident = np.eye(128, dtype=bf)
    tri = np.triu(np.ones((T, T), dtype=np.float32))  # [s, t]: 1 if s <= t
    mask = np.ascontiguousarray(np.tile(tri, (2, 8)).astype(bf))
    in_maps = []
    for c in range(N_CORES):
        shard = np.ascontiguousarray(
            x[c * B_CORE:(c + 1) * B_CORE].reshape(B_CORE * T, C)
        ).astype(np.float32)
        in_maps.append({
            "x": shard, "amat": amat, "wv": wv_bf,
            "ident": ident, "mask": mask,
        })
    return in_maps


def run(x, Wq, Wk, Wv, trace=False, **run_kwargs):
    from concourse import bass_utils

    if "nc" not in _cached:
        _cached["nc"] = _build_nc()
    nc = _cached["nc"]
    in_maps = _host_inputs(np.asarray(x), np.asarray(Wq),
                           np.asarray(Wk), np.asarray(Wv))
    res = bass_utils.run_bass_kernel_spmd(
        nc, in_maps, core_ids=list(range(N_CORES)), trace=trace, **run_kwargs
    )
    outs = [r["o"].reshape(B_CORE, T, H) for r in res.results]
    return np.concatenate(outs, axis=0), res


def kernel(x, Wq, Wk, Wv):
    out, _ = run(x, Wq, Wk, Wv, trace=False)
    return out


# revision 8
# speedup vs baseline: 1.5549x; 1.0246x over previous
"""Trainium2 Bass kernel: single-head causal attention, data-parallel over batch.

Problem: x [4096, 64, 128] f32, Wq/Wk/Wv [128, 64] f32.
  q,k,v = x @ W*;  scores = q k^T / sqrt(128); causal softmax; out = attn @ v.

Sharding: batch 4096 -> 8 cores x 512 batches. Each core loops over 32
super-tiles of 16 batches (1024 rows of x).

v2 pipeline (bf16 matmuls, fp32 PSUM):
  1. SWDGE DMA-cast loads x tile [128, 1024] f32->bf16 (cast in DMA).
  2. 8 PE transposes -> x^T in PSUM (bf16) -> SBUF.
  3. Y-pass: y^T = A^T x^T with A = (Wq/sqrt(C)) Wk^T precomputed on host
     (fuses the q- and k-projections: scores_b = x_b A x_b^T).
  4. V-pass: v pairs in native [s, h] layout (x^T pair as stationary).
  5. P3: scores^T_b = x_b y_b^T per batch: lhsT = xT_b, rhs = yT_b,
     2-way col-packed (K=128 full array).
  6. exp on ACT writes block-diag E pair slabs; multiplicative causal
     mask on GPSIMD (off-diag blocks stay zero from a one-time memset).
  7. P4: [O'|sums] = E_pair^T @ [V_pair|ones]: one M=128 matmul per
     2-batch pair (block-diagonal stationary).
  8. normalize: O = O' * recip(sums) via stride-0 broadcast tensor_tensor.
"""

import os
import numpy as np
import ml_dtypes
from contextlib import ExitStack

B, T, C, H = 4096, 64, 128, 64
N_CORES = 8
ST_B = 16                    # batches per super-tile
ROWS = ST_B * T              # 1024
B_CORE = B // N_CORES        # 512
N_ST = B_CORE // ST_B        # 32

_cached = {}


def _build_nc():
    import concourse.bass as bass
    import concourse.mybir as mybir
    import concourse.tile as tile
    from concourse import bacc

    F32 = mybir.dt.float32
    BF16 = mybir.dt.bfloat16

    nc = bacc.Bacc("TRN2", target_bir_lowering=False, debug=False)
    x_d = nc.dram_tensor("x", [B_CORE * T, C], F32, kind="ExternalInput").ap()
    a_d = nc.dram_tensor("amat", [C, C], BF16, kind="ExternalInput").ap()
    wv_d = nc.dram_tensor("wv", [C, H], BF16, kind="ExternalInput").ap()
    id_d = nc.dram_tensor("ident", [C, C], BF16, kind="ExternalInput").ap()
    mk_d = nc.dram_tensor("mask", [128, 512], BF16, kind="ExternalInput").ap()
    o_d = nc.dram_tensor("o", [B_CORE * T, H], F32, kind="ExternalOutput").ap()

    with tile.TileContext(nc) as tc, ExitStack() as ctx:
        sb = ctx.enter_context(tc.tile_pool(name="sb", bufs=3))
        ps = ctx.enter_context(tc.tile_pool(name="ps", bufs=1, space="PSUM"))
        psS = ctx.enter_context(tc.tile_pool(name="psS", bufs=2, space="PSUM"))
        psO = ctx.enter_context(tc.tile_pool(name="psO", bufs=1, space="PSUM"))
        cpool = ctx.enter_context(tc.tile_pool(name="const", bufs=1))

        a_sb = cpool.tile([C, C], BF16, tag="amat")
        wv_sb = cpool.tile([C, H], BF16, tag="wv")
        id_sb = cpool.tile([C, C], BF16, tag="id")
        mk_sb = cpool.tile([128, 512], BF16, tag="mk")
        nc.sync.dma_start(a_sb[:], a_d)
        nc.sync.dma_start(wv_sb[:], wv_d)
        nc.sync.dma_start(id_sb[:], id_d)
        nc.sync.dma_start(mk_sb[:], mk_d)

        xv = x_d.rearrange("(S n p) c -> S p n c", n=8, p=128)
        ov = o_d.rearrange("(S m par t) h -> S (par t) m h", m=8, par=2, t=64)

        for st in range(N_ST):
            # ---- SWDGE DMA-cast load: x tile f32 -> bf16
            x_bf = sb.tile([128, ROWS], BF16, tag="x_bf")
            nc.gpsimd.dma_start(
                x_bf[:].rearrange("p (n c) -> p n c", n=8), xv[st]
            )

            # ---- 8 PE transposes -> xT in PSUM (bf16), then copy to SBUF
            xT_ps = ps.tile([128, ROWS // 2], F32, tag="xT")
            xT_ps_bf = xT_ps[:].bitcast(BF16)
            for i in range(8):
                nc.tensor.transpose(
                    xT_ps_bf[:, 128 * i:128 * (i + 1)],
                    x_bf[:, 128 * i:128 * (i + 1)],
                    id_sb[:],
                )
            xT_sb = sb.tile([128, ROWS], BF16, tag="xT_sb")
            nc.vector.tensor_copy(xT_sb[:], xT_ps_bf)

            # ---- Y-pass: y^T = A^T @ x^T  [c2, rows]
            y_ps = ps.tile([128, ROWS], F32, tag="y")
            for b in range(2):
                nc.tensor.matmul(
                    y_ps[:, 512 * b:512 * b + 512],
                    a_sb[:],
                    xT_sb[:, 512 * b:512 * b + 512],
                    start=True, stop=True,
                )
            y_sb = sb.tile([128, ROWS], BF16, tag="y_sb")
            nc.scalar.copy(y_sb[:], y_ps[:])

            # ---- V-pass: v pairs (native [s,h]), xT pair as stationary
            v_ps = ps.tile([128, 512], F32, tag="v")
            for m in range(8):
                nc.tensor.matmul(
                    v_ps[:, 64 * m:64 * m + 64],
                    xT_sb[:, 128 * m:128 * m + 128],
                    wv_sb[:],
                    start=True, stop=True,
                )
            v_sb = sb.tile([128, 8 * 66], BF16, tag="v_sb")
            v_sb_v = v_sb[:].rearrange("p (m z) -> p m z", z=66)
            nc.vector.tensor_copy(
                v_sb_v[:, :, 0:64],
                v_ps[:].rearrange("p (m t) -> p m t", t=64),
            )
            nc.gpsimd.memset(v_sb_v[:, :, 64:65], 1.0)

            # ---- P3: scores^T_b = x_b y_b^T  (K=128, 2-way col-packed)
            sc_ps = psS.tile([128, 512], F32, tag="sc")
            for b in range(ST_B):
                m, half = b // 2, b % 2
                col = 128 * m + 64 * half
                nc.tensor.matmul(
                    sc_ps[64 * half:64 * half + 64, 64 * m:64 * m + 64],
                    xT_sb[:, col:col + 64],
                    y_sb[:, col:col + 64],
                    start=True, stop=True, skip_group_check=True,
                    tile_position=(0, 64 * half),
                )

            # ---- exp (ACT) then multiplicative causal mask (Vector)
            E_raw = sb.tile([128, 512], BF16, tag="Eraw")
            nc.scalar.activation(
                E_raw[:], sc_ps[:], mybir.ActivationFunctionType.Exp
            )
            E_sb = sb.tile([128, 512], BF16, tag="E")
            nc.gpsimd.tensor_tensor(
                out=E_sb[:], in0=E_raw[:], in1=mk_sb[:],
                op=mybir.AluOpType.mult,
            )

            # ---- P4: [O'|sums] = E_b^T @ [V_b|ones] per batch (quadrants)
            o_ps = psO.tile([128, 1024], F32, tag="o")
            for b in range(ST_B):
                m, half = b // 2, b % 2
                Hh = 64 * half
                off = 512 * (m // 4) + 65 * (m % 4)
                nc.tensor.matmul(
                    o_ps[Hh:Hh + 64, off:off + 65],
                    E_sb[Hh:Hh + 64, 64 * m:64 * m + 64],
                    v_sb[Hh:Hh + 64, 66 * m:66 * m + 65],
                    start=True, stop=True, skip_group_check=True,
                    tile_position=(Hh, Hh),
                )

            # ---- normalize: O = O' * recip(sums)
            opsv = o_ps[:].rearrange("p (B x) -> p B x", B=2)[:, :, 0:260]
            opsb = opsv.rearrange("p B (m z) -> p B m z", z=65)
            r_sb = sb.tile([128, 8], F32, tag="r")
            r_v = r_sb[:].rearrange("p (B m) -> p B m", B=2)
            nc.vector.reciprocal(r_v.unsqueeze(3), opsb[:, :, :, 64:65])
            o_sb = sb.tile([128, 512], F32, tag="o_sb")
            nc.vector.tensor_tensor(
                out=o_sb[:].rearrange("p (B m t) -> p B m t", B=2, t=64),
                in0=opsb[:, :, :, 0:64],
                in1=r_v.unsqueeze(3).broadcast_to((128, 2, 4, 64)),
                op=mybir.AluOpType.mult,
            )

            # ---- DMA out
            nc.sync.dma_start(ov[st], o_sb[:].rearrange("p (m h) -> p m h", h=64))

    nc.compile()
    return nc


def _host_inputs(x, Wq, Wk, Wv):
    bf = ml_dtypes.bfloat16
    amat = np.ascontiguousarray(
        ((Wq.astype(np.float32) * (C ** -0.5)) @ Wk.astype(np.float32).T)
        .astype(bf)
    )
    wv_bf = np.ascontiguousarray(Wv.astype(bf))
    ident = np.eye(128, dtype=bf)
    tri = np.triu(np.ones((T, T), dtype=np.float32))  # [s, t]: 1 if s <= t
    mask = np.ascontiguousarray(np.tile(tri, (2, 8)).astype(bf))
    in_maps = []
    for c in range(N_CORES):
        shard = np.ascontiguousarray(
            x[c * B_CORE:(c + 1) * B_CORE].reshape(B_CORE * T, C)
        ).astype(np.float32)
        in_maps.append({
            "x": shard, "amat": amat, "wv": wv_bf,
            "ident": ident, "mask": mask,
        })
    return in_maps


def run(x, Wq, Wk, Wv, trace=False, **run_kwargs):
    from concourse import bass_utils

    if "nc" not in _cached:
        _cached["nc"] = _build_nc()
    nc = _cached["nc"]
    in_maps = _host_inputs(np.asarray(x), np.asarray(Wq),
                           np.asarray(Wk), np.asarray(Wv))
    res = bass_utils.run_bass_kernel_spmd(
        nc, in_maps, core_ids=list(range(N_CORES)), trace=trace, **run_kwargs
    )
    outs = [r["o"].reshape(B_CORE, T, H) for r in res.results]
    return np.concatenate(outs, axis=0), res


def kernel(x, Wq, Wk, Wv):
    out, _ = run(x, Wq, Wk, Wv, trace=False)
    return out


# revision 10
# speedup vs baseline: 1.7361x; 1.1166x over previous
"""Trainium2 Bass kernel: single-head causal attention, data-parallel over batch.

Problem: x [4096, 64, 128] f32, Wq/Wk/Wv [128, 64] f32.
  q,k,v = x @ W*;  scores = q k^T / sqrt(128); causal softmax; out = attn @ v.

Sharding: batch 4096 -> 8 cores x 512 batches. Each core loops over 32
super-tiles of 16 batches (1024 rows of x).

v2 pipeline (bf16 matmuls, fp32 PSUM):
  1. SWDGE DMA-cast loads x tile [128, 1024] f32->bf16 (cast in DMA).
  2. 8 PE transposes -> x^T in PSUM (bf16) -> SBUF.
  3. Y-pass: y^T = A^T x^T with A = (Wq/sqrt(C)) Wk^T precomputed on host
     (fuses the q- and k-projections: scores_b = x_b A x_b^T).
  4. V-pass: v pairs in native [s, h] layout (x^T pair as stationary).
  5. P3: scores^T_b = x_b y_b^T per batch: lhsT = xT_b, rhs = yT_b,
     2-way col-packed (K=128 full array).
  6. exp on ACT writes block-diag E pair slabs; multiplicative causal
     mask on GPSIMD (off-diag blocks stay zero from a one-time memset).
  7. P4: [O'|sums] = E_pair^T @ [V_pair|ones]: one M=128 matmul per
     2-batch pair (block-diagonal stationary).
  8. normalize: O = O' * recip(sums) via stride-0 broadcast tensor_tensor.
"""

import os
import numpy as np
import ml_dtypes
from contextlib import ExitStack

B, T, C, H = 4096, 64, 128, 64
N_CORES = 8
ST_B = 16                    # batches per super-tile
ROWS = ST_B * T              # 1024
B_CORE = B // N_CORES        # 512
N_ST = B_CORE // ST_B        # 32

_cached = {}


def _build_nc():
    import concourse.bass as bass
    import concourse.mybir as mybir
    import concourse.tile as tile
    from concourse import bacc

    F32 = mybir.dt.float32
    BF16 = mybir.dt.bfloat16

    nc = bacc.Bacc("TRN2", target_bir_lowering=False, debug=False)
    x_d = nc.dram_tensor("x", [B_CORE * T, C], F32, kind="ExternalInput").ap()
    a_d = nc.dram_tensor("amat", [C, C], BF16, kind="ExternalInput").ap()
    wv_d = nc.dram_tensor("wv", [C, H], BF16, kind="ExternalInput").ap()
    id_d = nc.dram_tensor("ident", [C, C], BF16, kind="ExternalInput").ap()
    u_d = nc.dram_tensor("umat", [64, 128], BF16, kind="ExternalInput").ap()
    ni_d = nc.dram_tensor("negi", [64, 512], BF16, kind="ExternalInput").ap()
    o_d = nc.dram_tensor("o", [B_CORE * T, H], F32, kind="ExternalOutput").ap()

    with tile.TileContext(nc) as tc, ExitStack() as ctx:
        sb = ctx.enter_context(tc.tile_pool(name="sb", bufs=3))
        ps = ctx.enter_context(tc.tile_pool(name="ps", bufs=1, space="PSUM"))
        psS = ctx.enter_context(tc.tile_pool(name="psS", bufs=2, space="PSUM"))
        psO = ctx.enter_context(tc.tile_pool(name="psO", bufs=1, space="PSUM"))
        cpool = ctx.enter_context(tc.tile_pool(name="const", bufs=1))

        a_sb = cpool.tile([C, C], BF16, tag="amat")
        wv_sb = cpool.tile([C, H], BF16, tag="wv")
        id_sb = cpool.tile([C, C], BF16, tag="id")
        u_sb = cpool.tile([64, 128], BF16, tag="umat")
        ni_sb = cpool.tile([64, 512], BF16, tag="negi")
        nc.sync.dma_start(a_sb[:], a_d)
        nc.sync.dma_start(wv_sb[:], wv_d)
        nc.sync.dma_start(id_sb[:], id_d)
        nc.sync.dma_start(u_sb[:], u_d)
        nc.sync.dma_start(ni_sb[:], ni_d)

        xv = x_d.rearrange("(S n p) c -> S p n c", n=8, p=128)
        ov = o_d.rearrange("(S m par t) h -> S (par t) m h", m=8, par=2, t=64)

        for st in range(N_ST):
            # ---- SWDGE DMA-cast load: x tile f32 -> bf16
            x_bf = sb.tile([128, ROWS], BF16, tag="x_bf")
            nc.gpsimd.dma_start(
                x_bf[:].rearrange("p (n c) -> p n c", n=8), xv[st]
            )

            # ---- 8 PE transposes -> xT in PSUM (bf16), then copy to SBUF
            xT_ps = ps.tile([128, ROWS // 2], F32, tag="xT")
            xT_ps_bf = xT_ps[:].bitcast(BF16)
            for i in range(8):
                nc.tensor.transpose(
                    xT_ps_bf[:, 128 * i:128 * (i + 1)],
                    x_bf[:, 128 * i:128 * (i + 1)],
                    id_sb[:],
                )
            xT_sb = sb.tile([128, ROWS], BF16, tag="xT_sb")
            nc.vector.tensor_copy(xT_sb[:], xT_ps_bf)

            # ---- Y-pass: y^T = A^T @ x^T  [c2, rows]
            y_ps = ps.tile([128, ROWS], F32, tag="y")
            for b in range(2):
                nc.tensor.matmul(
                    y_ps[:, 512 * b:512 * b + 512],
                    a_sb[:],
                    xT_sb[:, 512 * b:512 * b + 512],
                    start=True, stop=True,
                )
            y_sb = sb.tile([128, ROWS], BF16, tag="y_sb")
            nc.scalar.copy(y_sb[:, 0:512], y_ps[:, 0:512])
            nc.scalar.copy(y_sb[:, 512:1024], y_ps[:, 512:1024])

            # ---- V-pass: v pairs (native [s,h]), xT pair as stationary
            v_ps = ps.tile([128, 512], F32, tag="v")
            for m in range(8):
                nc.tensor.matmul(
                    v_ps[:, 64 * m:64 * m + 64],
                    xT_sb[:, 128 * m:128 * m + 128],
                    wv_sb[:],
                    start=True, stop=True,
                )
            v_sb = sb.tile([128, 8 * 66], BF16, tag="v_sb")
            v_sb_v = v_sb[:].rearrange("p (m z) -> p m z", z=66)
            nc.vector.tensor_copy(
                v_sb_v[:, :, 0:64],
                v_ps[:].rearrange("p (m t) -> p m t", t=64),
            )
            nc.gpsimd.memset(v_sb_v[:, :, 64:65], 1.0)

            # ---- P3: causal bias first (start=True fills whole bank),
            #      then scores^T_b = x_b y_b^T accumulate (K=128, col-packed)
            sc_ps = psS.tile([128, 512], F32, tag="sc")
            nc.tensor.matmul(
                sc_ps[:, 0:512],
                u_sb[:],
                ni_sb[:],
                start=True, stop=False, skip_group_check=True,
                tile_position=(0, 0),
            )
            for b in range(ST_B):
                m, half = b // 2, b % 2
                col = 128 * m + 64 * half
                nc.tensor.matmul(
                    sc_ps[64 * half:64 * half + 64, 64 * m:64 * m + 64],
                    xT_sb[:, col:col + 64],
                    y_sb[:, col:col + 64],
                    start=False, stop=(b == ST_B - 1), skip_group_check=True,
                    tile_position=(0, 64 * half),
                )

            # ---- exp (ACT) -> E
            E_sb = sb.tile([128, 512], BF16, tag="E")
            nc.scalar.activation(
                E_sb[:], sc_ps[:], mybir.ActivationFunctionType.Exp
            )

            # ---- P4: [O'|sums] = E_b^T @ [V_b|ones] per batch (quadrants)
            o_ps = psO.tile([128, 1024], F32, tag="o")
            for b in range(ST_B):
                m, half = b // 2, b % 2
                Hh = 64 * half
                off = 512 * (m // 4) + 65 * (m % 4)
                nc.tensor.matmul(
                    o_ps[Hh:Hh + 64, off:off + 65],
                    E_sb[Hh:Hh + 64, 64 * m:64 * m + 64],
                    v_sb[Hh:Hh + 64, 66 * m:66 * m + 65],
                    start=True, stop=True, skip_group_check=True,
                    tile_position=(Hh, Hh),
                )

            # ---- normalize: O = O' * recip(sums)
            opsv = o_ps[:].rearrange("p (B x) -> p B x", B=2)[:, :, 0:260]
            opsb = opsv.rearrange("p B (m z) -> p B m z", z=65)
            r_sb = sb.tile([128, 8], F32, tag="r")
            r_v = r_sb[:].rearrange("p (B m) -> p B m", B=2)
            nc.vector.reciprocal(r_v.unsqueeze(3), opsb[:, :, :, 64:65])
            o_sb = sb.tile([128, 512], F32, tag="o_sb")
            nc.vector.tensor_tensor(
                out=o_sb[:].rearrange("p (B m t) -> p B m t", B=2, t=64),
                in0=opsb[:, :, :, 0:64],
                in1=r_v.unsqueeze(3).broadcast_to((128, 2, 4, 64)),
                op=mybir.AluOpType.mult,
            )

            # ---- DMA out
            nc.sync.dma_start(ov[st], o_sb[:].rearrange("p (m h) -> p m h", h=64))

    nc.compile()
    return nc


def _host_inputs(x, Wq, Wk, Wv):
    bf = ml_dtypes.bfloat16
    amat = np.ascontiguousarray(
        ((Wq.astype(np.float32) * (C ** -0.5)) @ Wk.astype(np.float32).T)
        .astype(bf)
    )
    wv_bf = np.ascontiguousarray(Wv.astype(bf))
    ident = np.eye(128, dtype=bf)
    u1 = np.triu(np.ones((T, T), dtype=np.float32), k=1)
    umat = np.ascontiguousarray(np.hstack([u1, u1]).astype(bf))
    negi = np.ascontiguousarray(
        (-30.0 * np.tile(np.eye(T, dtype=np.float32), (1, 8))).astype(bf))
    in_maps = []
    for c in range(N_CORES):
        shard = np.ascontiguousarray(
            x[c * B_CORE:(c + 1) * B_CORE].reshape(B_CORE * T, C)
        ).astype(np.float32)
        in_maps.append({
            "x": shard, "amat": amat, "wv": wv_bf,
            "ident": ident, "umat": umat, "negi": negi,
        })
    return in_maps


def run(x, Wq, Wk, Wv, trace=False, **run_kwargs):
    from concourse import bass_utils

    if "nc" not in _cached:
        _cached["nc"] = _build_nc()
    nc = _cached["nc"]
    in_maps = _host_inputs(np.asarray(x), np.asarray(Wq),
                           np.asarray(Wk), np.asarray(Wv))
    res = bass_utils.run_bass_kernel_spmd(
        nc, in_maps, core_ids=list(range(N_CORES)), trace=trace, **run_kwargs
    )
    outs = [r["o"].reshape(B_CORE, T, H) for r in res.results]
    return np.concatenate(outs, axis=0), res


def kernel(x, Wq, Wk, Wv):
    out, _ = run(x, Wq, Wk, Wv, trace=False)
    return out
